# revision 1
# baseline (speedup 1.0000x reference)
"""LIF layer (T=64, B=128, 2048->2048) on 8 trn2 NeuronCores.

Strategy: tensor-parallel over out_dim (each core owns 256 output
channels, sees the full x_seq). Per core:
  GEMM  cur[o, t*B+b] = sum_i W[o,i] * x[t,b,i]   (W stationary in PE)
  SCAN  64 sequential LIF steps on [128, 2, 128] tiles (DVE), reading
        cur straight out of PSUM.
Bias is folded away via the change of variable u = mem - b/(1-decay),
turning the per-step bias add into a per-channel spike threshold.

Host-side prep (not on HW): transpose x to [I, T*B], slice/transpose W,
precompute threshold/init tiles, final output is a cheap transpose+concat.
"""

import math
import os

import numpy as np

import concourse.bacc as bacc
import concourse.bass as bass
import concourse.mybir as mybir
import concourse.tile as tile
from concourse import bass_utils

# Problem constants (hardcoded per contract)
T, B, I, O = 64, 128, 2048, 2048
N_CORES = 8
OL = O // N_CORES          # 256 out-channels per core
TB = T * B                 # 8192 rows
KT = I // 128              # 16 k-tiles
NPB_COLS = 1024            # tb-columns per block (= 8 timesteps)
N_NPB = TB // NPB_COLS     # 8 blocks
MM_N = 512                 # moving free dim per matmul (psum bank)
TAU, THR = 2.0, 1.0
DECAY = math.exp(-1.0 / TAU)

F32 = mybir.dt.float32
ALU = mybir.AluOpType

# GEMM precision mode: "fp32" (exact, 4 cyc/row), "f32r" (fp22 inputs,
# 1 cyc/row), "bf16x3" (3-pass hi/lo split, ~exact, 3 cyc/row)
MODE = os.environ.get("LIF_MODE", "bf16x3")

_cache = {}


def _build_nc(mode):
    nc = bacc.Bacc(trn_type="TRN2", target_bir_lowering=False)

    mm_dt = {"fp32": F32, "f32r": mybir.dt.float32r, "bf16x3": mybir.dt.bfloat16}[mode]

    # DRAM I/O. "stages": list of (x_dram, [w_variant_idx,...]) — each stage
    # loads its x tile once and runs matmuls against each listed w variant,
    # all accumulating into the same PSUM group.
    if mode == "bf16x3":
        xT_h = nc.dram_tensor("xT_h", [I, TB], mybir.dt.bfloat16, kind="ExternalInput")
        xT_l = nc.dram_tensor("xT_l", [I, TB], mybir.dt.bfloat16, kind="ExternalInput")
        n_wv = 2
        stages = [(xT_h, [0, 1]), (xT_l, [0])]   # xh@wh + xh@wl + xl@wh
    else:
        xT = nc.dram_tensor("xT", [I, TB], F32, kind="ExternalInput")
        n_wv = 1
        stages = [(xT, [0])]
    # weights pre-packed on host to w_all's exact SBUF layout -> one fast DMA
    w_packed = nc.dram_tensor("w_packed", [128, n_wv, KT, 2, 128], mm_dt,
                              kind="ExternalInput")
    n_mm_grp = sum(len(wvs) for _, wvs in stages) * KT  # accum group length
    thr_d = nc.dram_tensor("thr", [128, 2, 128], F32, kind="ExternalInput")
    u0_d = nc.dram_tensor("u0", [128, 2, 128], F32, kind="ExternalInput")
    out_d = nc.dram_tensor("out", [128, 2, T, B], F32, kind="ExternalOutput")

    with tile.TileContext(nc) as tc:
        with (
            tc.tile_pool(name="wpool", bufs=1) as wpool,
            tc.tile_pool(name="xpool", bufs=8) as xpool,
            tc.tile_pool(name="state", bufs=1) as state,
            tc.tile_pool(name="spkpool", bufs=4) as spkpool,
            tc.tile_pool(name="psum", bufs=8, space="PSUM") as psum_pool,
        ):
            # Preload weight tiles, one contiguous DMA per variant (gpsimd
            # queue, parallel to the x-prefetch on the sync queue); first
            # matmul only gates on variant 0.
            w_all = wpool.tile([128, n_wv, KT, 2, 128], mm_dt)
            for wv in range(n_wv):
                nc.gpsimd.dma_start(w_all[:, wv], w_packed[:, wv])

            # Persistent state tiles
            u = state.tile([128, 2, 128], F32)
            thr_t = state.tile([128, 2, 128], F32)
            nc.gpsimd.dma_start(u[:], u0_d[:])
            nc.gpsimd.dma_start(thr_t[:], thr_d[:])

            # col-blocks: 1024-wide except the last 1024 split in two, so the
            # final exposed scan (after the last matmul) is only 4 steps
            blocks = [(i * NPB_COLS, NPB_COLS) for i in range(N_NPB - 1)]
            blocks += [(TB - 1024, 512), (TB - 512, 512)]
            for bi, (cs, ncols) in enumerate(blocks):
                n_nn = ncols // MM_N
                # one psum tile per (ot, nn): [128, 512] fp32 = 1 bank
                ps = [[psum_pool.tile([128, MM_N], F32, tag="ps", name=f"ps_{bi}_{ot}_{nn}")
                       for nn in range(n_nn)] for ot in range(2)]
                mm_i = 0
                for x_src, wvs in stages:
                    for k in range(KT):
                        xt = xpool.tile([128, NPB_COLS], mm_dt, tag="xt",
                                        name=f"xt_{bi}_{k}")
                        nc.sync.dma_start(
                            xt[:, :ncols],
                            x_src[k * 128:(k + 1) * 128, cs:cs + ncols],
                        )
                        for wv in wvs:
                            mm_i += 1
                            for ot in range(2):
                                for nn in range(n_nn):
                                    nc.tensor.matmul(
                                        ps[ot][nn][:],
                                        w_all[:, wv, k, ot, :],
                                        xt[:, nn * MM_N:(nn + 1) * MM_N],
                                        start=(mm_i == 1),
                                        stop=(mm_i == n_mm_grp),
                                    )

                # LIF steps consuming this block's PSUM
                for tl in range(ncols // 128):
                    t = (cs // 128) + tl  # global timestep
                    nn, off = tl // 4, (tl % 4) * 128
                    nc.vector.tensor_scalar_mul(u[:], u[:], DECAY)
                    nc.vector.tensor_tensor(
                        u[:, 0, :], u[:, 0, :], ps[0][nn][:, off:off + 128], op=ALU.add)
                    nc.vector.tensor_tensor(
                        u[:, 1, :], u[:, 1, :], ps[1][nn][:, off:off + 128], op=ALU.add)
                    spk = spkpool.tile([128, 2, 128], F32, tag="spk")
                    nc.vector.tensor_tensor(spk[:], u[:], thr_t[:], op=ALU.is_gt)
                    nc.vector.tensor_tensor(u[:], u[:], spk[:], op=ALU.subtract)
                    nc.gpsimd.dma_start(out_d[:, :, t, :], spk[:])

    nc.compile()
    return nc


def _get_nc(mode):
    if mode not in _cache:
        _cache[mode] = _build_nc(mode)
    return _cache[mode]


def kernel(x_seq: np.ndarray, W: np.ndarray, b: np.ndarray) -> np.ndarray:
    mode = MODE
    nc = _get_nc(mode)

    x = np.ascontiguousarray(x_seq.reshape(TB, I), dtype=np.float32)
    xT = np.ascontiguousarray(x.T)  # [I, TB]

    if mode == "bf16x3":
        import ml_dtypes
        xT_h = xT.astype(ml_dtypes.bfloat16)
        xT_l = (xT - xT_h.astype(np.float32)).astype(ml_dtypes.bfloat16)

    in_maps = []
    for c in range(N_CORES):
        w_c = W[c * OL:(c + 1) * OL, :].astype(np.float32)      # [OL, I]
        wTc = np.ascontiguousarray(w_c.T)                       # [I, OL]
        b_c = b[c * OL:(c + 1) * OL].astype(np.float32)         # [OL]
        shift = b_c / (1.0 - DECAY)
        thr = (THR - shift).reshape(2, 128).transpose(1, 0)     # [128(op), 2(ot)]
        u0 = (-shift).reshape(2, 128).transpose(1, 0)
        thr_tile = np.ascontiguousarray(
            np.broadcast_to(thr[:, :, None], (128, 2, 128)), dtype=np.float32)
        u0_tile = np.ascontiguousarray(
            np.broadcast_to(u0[:, :, None], (128, 2, 128)), dtype=np.float32)
        m = {"thr": thr_tile, "u0": u0_tile}

        def pack_w(wt):  # [I, OL] -> [128(p), KT, 2(ot), 128(f)]
            return wt.reshape(KT, 128, 2, 128).transpose(1, 0, 2, 3)

        if mode == "bf16x3":
            wTc_h = wTc.astype(ml_dtypes.bfloat16)
            wTc_l = (wTc - wTc_h.astype(np.float32)).astype(ml_dtypes.bfloat16)
            wp = np.ascontiguousarray(
                np.stack([pack_w(wTc_h), pack_w(wTc_l)], axis=1))
            m.update(xT_h=xT_h, xT_l=xT_l, w_packed=wp)
        else:
            wp = np.ascontiguousarray(pack_w(wTc)[:, None])
            m.update(xT=xT, w_packed=wp)
        in_maps.append(m)

    res = bass_utils.run_bass_kernel_spmd(nc, in_maps, core_ids=list(range(N_CORES)))
    global LAST_RESULT
    LAST_RESULT = res

    # Assemble: out_c[op, ot, t, b] -> [t, b, ot*128+op]; concat over cores
    parts = []
    for c in range(N_CORES):
        oc = res.results[c]["out"]  # [128, 2, T, B]
        parts.append(oc.transpose(2, 3, 1, 0).reshape(T, B, 2 * 128))
    return np.ascontiguousarray(np.concatenate(parts, axis=2))


LAST_RESULT = None



# revision 5
# speedup vs baseline: 1.6362x; 1.6362x over previous
"""LIF layer (T=64, B=128, 2048->2048) on 8 trn2 NeuronCores.

Strategy: tensor-parallel over out_dim (each core owns 256 output
channels, sees the full x_seq). Per core:
  GEMM  cur[o, t*B+b] = sum_i W[o,i] * x[t,b,i]   (W stationary in PE)
  SCAN  64 sequential LIF steps on [128, 2, 128] tiles (DVE), reading
        cur straight out of PSUM.
Bias is folded away via the change of variable u = mem - b/(1-decay),
turning the per-step bias add into a per-channel spike threshold.

Host-side prep (not on HW): transpose x to [I, T*B], slice/transpose W,
precompute threshold/init tiles, final output is a cheap transpose+concat.
"""

import math
import os

import numpy as np

import concourse.bacc as bacc
import concourse.bass as bass
import concourse.mybir as mybir
import concourse.tile as tile
from concourse import bass_utils

# Problem constants (hardcoded per contract)
T, B, I, O = 64, 128, 2048, 2048
N_CORES = 8
OL = O // N_CORES          # 256 out-channels per core
TB = T * B                 # 8192 rows
KT = I // 128              # 16 k-tiles
NPB_COLS = 1024            # tb-columns per block (= 8 timesteps)
N_NPB = TB // NPB_COLS     # 8 blocks
MM_N = 512                 # moving free dim per matmul (psum bank)
TAU, THR = 2.0, 1.0
DECAY = math.exp(-1.0 / TAU)

F32 = mybir.dt.float32
ALU = mybir.AluOpType

# GEMM precision mode: "fp32" (exact, 4 cyc/row), "f32r" (fp22 inputs,
# 1 cyc/row), "bf16x3" (3-pass hi/lo split, ~exact, 3 cyc/row)
MODE = os.environ.get("LIF_MODE", "bf16x3")
# Host-side RNE pre-rounding to this many mantissa bits for f32r inputs
# (0 = off). The PE truncates f32r inputs to FP22; pre-rounding with RNE
# halves the quantization error and removes the truncation bias.
RNE_BITS = int(os.environ.get("LIF_RNE", "0"))

_cache = {}


def _rne(a: np.ndarray, mant_bits: int) -> np.ndarray:
    """Round fp32 array to mant_bits mantissa bits with round-to-nearest-even."""
    if mant_bits <= 0 or mant_bits >= 23:
        return a
    drop = 23 - mant_bits
    u = a.astype(np.float32).view(np.uint32)
    lsb = (u >> drop) & 1
    u = u + ((1 << (drop - 1)) - 1) + lsb
    u &= np.uint32(0xFFFFFFFF) ^ np.uint32((1 << drop) - 1)
    return u.view(np.float32)


def _build_nc(mode):
    nc = bacc.Bacc(trn_type="TRN2", target_bir_lowering=False)

    mm_dt = {"fp32": F32, "f32r": mybir.dt.float32r, "bf16x3": mybir.dt.bfloat16}[mode]

    # DRAM I/O. "stages": list of (x_dram, [w_variant_idx,...]) — each stage
    # loads its x tile once and runs matmuls against each listed w variant,
    # all accumulating into the same PSUM group.
    if mode == "bf16x3":
        xT_h = nc.dram_tensor("xT_h", [I, TB], mybir.dt.bfloat16, kind="ExternalInput")
        xT_l = nc.dram_tensor("xT_l", [I, TB], mybir.dt.bfloat16, kind="ExternalInput")
        n_wv = 2
        stages = [(xT_h, [0, 1]), (xT_l, [0])]   # xh@wh + xh@wl + xl@wh
    else:
        xT = nc.dram_tensor("xT", [I, TB], mm_dt, kind="ExternalInput")
        n_wv = 1
        stages = [(xT, [0])]
    # weights pre-packed on host to w_all's exact SBUF layout -> one fast DMA
    w_packed = nc.dram_tensor("w_packed", [128, n_wv, KT, 2, 128], mm_dt,
                              kind="ExternalInput")
    n_mm_grp = sum(len(wvs) for _, wvs in stages) * KT  # accum group length
    thr_d = nc.dram_tensor("thr", [128, 2, 128], F32, kind="ExternalInput")
    u0_d = nc.dram_tensor("u0", [128, 2, 128], F32, kind="ExternalInput")
    out_d = nc.dram_tensor("out", [128, 2, T, B], F32, kind="ExternalOutput")

    with tile.TileContext(nc) as tc:
        with (
            tc.tile_pool(name="wpool", bufs=1) as wpool,
            tc.tile_pool(name="xpool", bufs=8) as xpool,
            tc.tile_pool(name="state", bufs=1) as state,
            tc.tile_pool(name="spkpool", bufs=4) as spkpool,
            tc.tile_pool(name="psum", bufs=8, space="PSUM") as psum_pool,
        ):
            # Preload weight tiles, one contiguous DMA per variant (gpsimd
            # queue, parallel to the x-prefetch on the sync queue); first
            # matmul only gates on variant 0.
            w_all = wpool.tile([128, n_wv, KT, 2, 128], mm_dt)
            for wv in range(n_wv):
                nc.gpsimd.dma_start(w_all[:, wv], w_packed[:, wv])

            # Persistent state tiles
            u = state.tile([128, 2, 128], F32)
            thr_t = state.tile([128, 2, 128], F32)
            nc.gpsimd.dma_start(u[:], u0_d[:])
            nc.gpsimd.dma_start(thr_t[:], thr_d[:])

            # col-blocks: 1024-wide except the last 1024 split in two, so the
            # final exposed scan (after the last matmul) is only 4 steps
            blocks = [(i * NPB_COLS, NPB_COLS) for i in range(N_NPB - 1)]
            blocks += [(TB - 1024, 512), (TB - 512, 512)]
            for bi, (cs, ncols) in enumerate(blocks):
                n_nn = ncols // MM_N
                # one psum tile per (ot, nn): [128, 512] fp32 = 1 bank
                ps = [[psum_pool.tile([128, MM_N], F32, tag="ps", name=f"ps_{bi}_{ot}_{nn}")
                       for nn in range(n_nn)] for ot in range(2)]
                mm_i = 0
                for x_src, wvs in stages:
                    for k in range(KT):
                        xt = xpool.tile([128, NPB_COLS], mm_dt, tag="xt",
                                        name=f"xt_{bi}_{k}")
                        nc.sync.dma_start(
                            xt[:, :ncols],
                            x_src[k * 128:(k + 1) * 128, cs:cs + ncols],
                        )
                        for wv in wvs:
                            mm_i += 1
                            for ot in range(2):
                                for nn in range(n_nn):
                                    nc.tensor.matmul(
                                        ps[ot][nn][:],
                                        w_all[:, wv, k, ot, :],
                                        xt[:, nn * MM_N:(nn + 1) * MM_N],
                                        start=(mm_i == 1),
                                        stop=(mm_i == n_mm_grp),
                                    )

                # LIF steps consuming this block's PSUM
                for tl in range(ncols // 128):
                    t = (cs // 128) + tl  # global timestep
                    nn, off = tl // 4, (tl % 4) * 128
                    nc.vector.tensor_scalar_mul(u[:], u[:], DECAY)
                    nc.vector.tensor_tensor(
                        u[:, 0, :], u[:, 0, :], ps[0][nn][:, off:off + 128], op=ALU.add)
                    nc.vector.tensor_tensor(
                        u[:, 1, :], u[:, 1, :], ps[1][nn][:, off:off + 128], op=ALU.add)
                    spk = spkpool.tile([128, 2, 128], F32, tag="spk")
                    nc.vector.tensor_tensor(spk[:], u[:], thr_t[:], op=ALU.is_gt)
                    nc.vector.tensor_tensor(u[:], u[:], spk[:], op=ALU.subtract)
                    nc.gpsimd.dma_start(out_d[:, :, t, :], spk[:])

    nc.compile()
    return nc


def _get_nc(mode):
    if mode not in _cache:
        _cache[mode] = _build_nc(mode)
    return _cache[mode]


def kernel(x_seq: np.ndarray, W: np.ndarray, b: np.ndarray) -> np.ndarray:
    mode = MODE
    nc = _get_nc(mode)

    x = np.ascontiguousarray(x_seq.reshape(TB, I), dtype=np.float32)
    xT = np.ascontiguousarray(x.T)  # [I, TB]
    if mode == "f32r":
        xT = _rne(xT, RNE_BITS)

    if mode == "bf16x3":
        import ml_dtypes
        xT_h = xT.astype(ml_dtypes.bfloat16)
        xT_l = (xT - xT_h.astype(np.float32)).astype(ml_dtypes.bfloat16)

    in_maps = []
    for c in range(N_CORES):
        w_c = W[c * OL:(c + 1) * OL, :].astype(np.float32)      # [OL, I]
        wTc = np.ascontiguousarray(w_c.T)                       # [I, OL]
        b_c = b[c * OL:(c + 1) * OL].astype(np.float32)         # [OL]
        shift = b_c / (1.0 - DECAY)
        thr = (THR - shift).reshape(2, 128).transpose(1, 0)     # [128(op), 2(ot)]
        u0 = (-shift).reshape(2, 128).transpose(1, 0)
        thr_tile = np.ascontiguousarray(
            np.broadcast_to(thr[:, :, None], (128, 2, 128)), dtype=np.float32)
        u0_tile = np.ascontiguousarray(
            np.broadcast_to(u0[:, :, None], (128, 2, 128)), dtype=np.float32)
        m = {"thr": thr_tile, "u0": u0_tile}

        def pack_w(wt):  # [I, OL] -> [128(p), KT, 2(ot), 128(f)]
            return wt.reshape(KT, 128, 2, 128).transpose(1, 0, 2, 3)

        if mode == "bf16x3":
            wTc_h = wTc.astype(ml_dtypes.bfloat16)
            wTc_l = (wTc - wTc_h.astype(np.float32)).astype(ml_dtypes.bfloat16)
            wp = np.ascontiguousarray(
                np.stack([pack_w(wTc_h), pack_w(wTc_l)], axis=1))
            m.update(xT_h=xT_h, xT_l=xT_l, w_packed=wp)
        else:
            wp = np.ascontiguousarray(pack_w(_rne(wTc, RNE_BITS))[:, None])
            m.update(xT=xT, w_packed=wp)
        in_maps.append(m)

    res = bass_utils.run_bass_kernel_spmd(nc, in_maps, core_ids=list(range(N_CORES)))
    global LAST_RESULT
    LAST_RESULT = res

    # Assemble: out_c[op, ot, t, b] -> [t, b, ot*128+op]; concat over cores
    parts = []
    for c in range(N_CORES):
        oc = res.results[c]["out"]  # [128, 2, T, B]
        parts.append(oc.transpose(2, 3, 1, 0).reshape(T, B, 2 * 128))
    return np.ascontiguousarray(np.concatenate(parts, axis=2))


LAST_RESULT = None



# revision 7
# speedup vs baseline: 1.9900x; 1.2163x over previous
"""LIF layer (T=64, B=128, 2048->2048) on 8 trn2 NeuronCores.

Sharding: 2-way over out_dim x 4-way over batch. Each core owns
O_loc=1024 output channels (8 chunks of 128) and B_loc=32 batch rows,
so x traffic per core is 16MB (vs 64MB for pure out_dim sharding) and
the kernel stays PE-bound at the f32r roofline.

GEMM: single-pass float32r (PE rounds inputs to ~fp22; 1 cycle/row for
moving free dim >= 256). Inputs are pre-rounded RNE to 11 mantissa bits
on the host (measured bit-identical to the HW rounding, documents the
precision contract). Per 8-timestep block, psum holds [128, 8 chunks,
256 cols] = 4 banks, double-buffered across blocks.

Scan: z-space reformulation removes the per-step decay multiply. Host
prescales x columns of step tau (within a block) by s_tau = d^-(tau+1);
bias and the -1 threshold shift enter through a rank-1 17th matmul
(bias row). With the sign-flipped state ybar = -d^-tau*(mem-1), one LIF
step is 3 DVE ops:
    A: ybar -= G          (psum read)
    B: spk = ybar < 0     (exact {0,1} spikes)
    C: ybar = spk*s_tau + ybar   (fused scalar_tensor_tensor)
and ybar *= d^8 once per block. Spikes DMA out per step.
"""

import math

import numpy as np

import concourse.bacc as bacc
import concourse.bass as bass
import concourse.mybir as mybir
import concourse.tile as tile
from concourse import bass_utils

# Problem constants (hardcoded per contract)
T, B, I, O = 64, 128, 2048, 2048
N_CORES = 8
OC_SHARD, BC_SHARD = 2, 4          # out_dim x batch sharding grid
O_LOC = O // OC_SHARD              # 1024 channels per core
B_LOC = B // BC_SHARD              # 32 batch rows per core
N_CHUNK = O_LOC // 128             # 8 stationary chunks
KT = I // 128                      # 16 k-tiles
STEPS_PER_BLK = 8                  # timesteps per psum block
N_BLK = T // STEPS_PER_BLK         # 8 blocks
COLS = STEPS_PER_BLK * B_LOC       # 256 moving columns per block
TAU_C, THR = 2.0, 1.0
DECAY = math.exp(-1.0 / TAU_C)
SCALES = [DECAY ** -(t + 1) for t in range(STEPS_PER_BLK)]

F32 = mybir.dt.float32
F32R = mybir.dt.float32r
ALU = mybir.AluOpType

MODE = "f32r_o2b4"

_cache = {}


def _rne(a: np.ndarray, mant_bits: int = 11) -> np.ndarray:
    """Round fp32 array to mant_bits mantissa bits, round-to-nearest-even."""
    drop = 23 - mant_bits
    u = np.ascontiguousarray(a, dtype=np.float32).view(np.uint32)
    lsb = (u >> drop) & 1
    u = u + ((1 << (drop - 1)) - 1) + lsb
    u &= np.uint32(0xFFFFFFFF) ^ np.uint32((1 << drop) - 1)
    return u.view(np.float32)


def _build_nc():
    nc = bacc.Bacc(trn_type="TRN2", target_bir_lowering=False)

    # DRAM I/O (per core). x_packed[k, blk] is a contiguous [128, 256]
    # tile: host-transposed, column-prescaled by s_tau, RNE-rounded.
    x_d = nc.dram_tensor("x_packed", [KT, N_BLK, 128, COLS], F32R,
                         kind="ExternalInput")
    w_d = nc.dram_tensor("w_packed", [128, KT, N_CHUNK, 128], F32R,
                         kind="ExternalInput")
    wb_d = nc.dram_tensor("wb", [1, N_CHUNK, 128], F32R, kind="ExternalInput")
    xb_d = nc.dram_tensor("xb", [1, COLS], F32R, kind="ExternalInput")
    out_d = nc.dram_tensor("out", [128, T, N_CHUNK, B_LOC], F32,
                           kind="ExternalOutput")

    with tile.TileContext(nc) as tc:
        with (
            tc.tile_pool(name="wpool", bufs=1) as wpool,
            tc.tile_pool(name="xpool", bufs=6) as xpool,
            tc.tile_pool(name="state", bufs=1) as state,
            tc.tile_pool(name="spkpool", bufs=4) as spkpool,
            tc.tile_pool(name="psum", bufs=2, space="PSUM") as psum_pool,
        ):
            # Resident weights: per-k DMAs so the first matmuls can start
            # before the whole 8MB lands. gpsimd queue, parallel to x on
            # sync/scalar queues.
            w_all = wpool.tile([128, KT, N_CHUNK, 128], F32R)
            for k in range(KT):
                nc.gpsimd.dma_start(w_all[:, k], w_d[:, k])
            wb_t = wpool.tile([1, N_CHUNK, 128], F32R)
            nc.gpsimd.dma_start(wb_t[:], wb_d[:])
            xb_t = wpool.tile([1, COLS], F32R)
            nc.gpsimd.dma_start(xb_t[:], xb_d[:])

            # State: ybar = -d^-tau (mem - 1); mem_0 = 0 -> ybar = 1.
            ybar = state.tile([128, N_CHUNK, B_LOC], F32)
            nc.vector.memset(ybar[:], 1.0)

            for bi in range(N_BLK):
                ps = psum_pool.tile([128, N_CHUNK, COLS], F32, tag="ps",
                                    name=f"ps_{bi}")
                for k in range(KT):
                    xt = xpool.tile([128, COLS], F32R, tag="xt",
                                    name=f"xt_{bi}_{k}")
                    eng = nc.sync if k % 2 == 0 else nc.scalar
                    eng.dma_start(xt[:], x_d[k, bi])
                    for c in range(N_CHUNK):
                        # start=True clears has_written for the WHOLE bank;
                        # chunks are packed 2 per bank, so only the first
                        # chunk in each bank may clear. The second chunk's
                        # k=0 write lands on cleared bits and overwrites.
                        nc.tensor.matmul(
                            ps[:, c, :], w_all[:, k, c, :], xt[:],
                            start=(k == 0 and c % 2 == 0), stop=False,
                        )
                # Rank-1 bias row closes each chunk's accumulation group:
                # adds s_tau * (b_o + d - 1) to every column.
                for c in range(N_CHUNK):
                    nc.tensor.matmul(
                        ps[:, c, :], wb_t[:, c, :], xb_t[:],
                        start=False, stop=True,
                    )

                # LIF scan consuming this block's psum
                for tau in range(STEPS_PER_BLK):
                    t = bi * STEPS_PER_BLK + tau
                    g = ps[:, :, tau * B_LOC:(tau + 1) * B_LOC]
                    nc.vector.tensor_tensor(ybar[:], ybar[:], g,
                                            op=ALU.subtract)
                    spk = spkpool.tile([128, N_CHUNK, B_LOC], F32, tag="spk")
                    nc.vector.tensor_scalar(spk[:], ybar[:], 0.0, None,
                                            op0=ALU.is_lt)
                    nc.vector.scalar_tensor_tensor(
                        ybar[:], spk[:], SCALES[tau], ybar[:],
                        op0=ALU.mult, op1=ALU.add,
                    )
                    nc.gpsimd.dma_start(out_d[:, t], spk[:])
                if bi + 1 < N_BLK:
                    nc.vector.tensor_scalar_mul(ybar[:], ybar[:],
                                                DECAY ** STEPS_PER_BLK)

    nc.compile()
    return nc


def _get_nc():
    if "nc" not in _cache:
        _cache["nc"] = _build_nc()
    return _cache["nc"]


def kernel(x_seq: np.ndarray, W: np.ndarray, b: np.ndarray) -> np.ndarray:
    nc = _get_nc()

    x_seq = np.ascontiguousarray(x_seq, dtype=np.float32)
    col_scale = np.array([SCALES[t % STEPS_PER_BLK] for t in range(T)],
                         dtype=np.float32)

    # Per-batch-shard x: [KT, N_BLK, 128, COLS], prescaled + RNE'd.
    x_parts = []
    for bc in range(BC_SHARD):
        xs = x_seq[:, bc * B_LOC:(bc + 1) * B_LOC, :]      # [T, B_LOC, I]
        xs = xs * col_scale[:, None, None]
        xp = xs.transpose(2, 0, 1)                         # [I, T, B_LOC]
        xp = xp.reshape(KT, 128, N_BLK, STEPS_PER_BLK * B_LOC)
        xp = np.ascontiguousarray(xp.transpose(0, 2, 1, 3))
        x_parts.append(_rne(xp))

    # Per-out-shard weights: [128(ip), KT, N_CHUNK, 128(of)], RNE'd.
    w_parts, wb_parts = [], []
    for oc in range(OC_SHARD):
        w_oc = W[oc * O_LOC:(oc + 1) * O_LOC, :].astype(np.float32)
        wT = _rne(w_oc.T)                                  # [I, O_LOC]
        wp = wT.reshape(KT, 128, N_CHUNK, 128).transpose(1, 0, 2, 3)
        w_parts.append(np.ascontiguousarray(wp))
        wb = b[oc * O_LOC:(oc + 1) * O_LOC].astype(np.float32) + DECAY - 1.0
        wb_parts.append(_rne(wb.reshape(1, N_CHUNK, 128)))

    xb = np.repeat(np.array(SCALES, dtype=np.float32), B_LOC).reshape(1, COLS)
    xb = _rne(xb)

    in_maps = []
    for c in range(N_CORES):
        oc, bc = divmod(c, BC_SHARD)
        in_maps.append({
            "x_packed": x_parts[bc],
            "w_packed": w_parts[oc],
            "wb": wb_parts[oc],
            "xb": xb,
        })

    res = bass_utils.run_bass_kernel_spmd(nc, in_maps, core_ids=list(range(N_CORES)))
    global LAST_RESULT
    LAST_RESULT = res

    # Assemble: out_c[op, t, chunk, beta] -> [t, b, o]
    result = np.empty((T, B, O), dtype=np.float32)
    for c in range(N_CORES):
        oc, bc = divmod(c, BC_SHARD)
        o_part = res.results[c]["out"]                     # [128, T, 8, 32]
        part = o_part.transpose(1, 3, 2, 0).reshape(T, B_LOC, O_LOC)
        result[:, bc * B_LOC:(bc + 1) * B_LOC,
               oc * O_LOC:(oc + 1) * O_LOC] = part
    return result


LAST_RESULT = None
